# revision 1
# baseline (speedup 1.0000x reference)
"""Trainium2 Bass kernel for nn_DifferentiableStarPlanner.

Algorithm notes (validated bitwise vs the reference in numpy):

  * The reference's open/close/pool computations never feed the returned
    tensor: the output is exactly NUM_SWEEPS Jacobi sweeps of a 9-channel
    min-plus stencil  g <- min(g, min_c(shift_c(g) + cmap_c))  with
    g0 = 1e7 everywhere except the start cell.
  * Only the start bounding box inflated by NUM_SWEEPS (clipped) can change
    from 1e7: a 113x113 corner here.  Edge-replicate padding is replaced by
    1e7 guard cells (provably never the argmin), the center channel by a
    pure-copy identity channel.
  * Per sweep only cells within t steps of the start can change, so all
    per-sweep work is windowed to the active wavefront.

Device mapping (one NeuronCore; all 8 cores run identical replicas).
The state alternates orientation every sweep; every neighbor shift is ONE
TensorEngine transpose-mode matmul (pure routing, bit-exact):

  * transpose-mode semantics: out[m, j] = lhsT[perm(j), m].  The stationary
    lhsT is the state sliced [0:K] (PE requires base partition 0); perm is a
    K x K cyclic permutation encoding the +-1 row shift (identity for the
    -1 shift); the col shift is the free-dim base of the lhsT slice.  INF
    guard partitions/columns at both ends of the state supply the domain
    boundary; cyclic wraps land in junk output columns beyond the window.
  * K = (window_hi + 3) rounded up to a multiple of 8 (a handful of distinct
    cyclic matrices, shipped in the packed input).  Matmul cost scales with
    output free size = K, so shift AND cmap-preload matmuls track the
    wavefront instead of paying full width every sweep.
  * Accumulation onto the cmap preload uses PSUM first-touch-overwrite
    semantics; the center region is never preloaded so the identity channel
    injects g itself.
  * The 9-way min reduce is split by columns between the DVE (one
    tensor_reduce) and the Pool/gpsimd engine (a 4-instruction min tree),
    halving the reduce latency on the critical path.
  * cmap for sweep t+1 is preloaded into the other PSUM bank set during
    sweep t, overlapped with the reduce.
"""
import sys
import os
import numpy as np

for _p in ("/opt/trn_rl_repo", "/root/.axon_site/_ro/trn_rl_repo"):
    if os.path.isdir(_p) and _p not in sys.path:
        sys.path.insert(0, _p)

import concourse.bass as bass
import concourse.bacc as bacc
import concourse.mybir as mybir
from concourse import tile
from concourse.bass_utils import run_bass_kernel_spmd

F32 = mybir.dt.float32
F32R = mybir.dt.float32r


def _rc(ap):
    return ap.bitcast(F32R) if USE_F32R else ap
ALU = mybir.AluOpType
AXL = mybir.AxisListType
ACTF = mybir.ActivationFunctionType

USE_F32R = os.environ.get("K_F32R", "") != ""
USE_POOL = os.environ.get("K_POOL", "") != ""

INF = np.float32(1.0e7)
OC = float(np.float32(10000.0))
EPS_F = np.float32(1e-12)
NUM_SWEEPS = 80
N_CORES = 8

# channels: (dy, dx), center excluded
CHANNELS = [(dy, dx) for dy in (-1, 0, 1) for dx in (-1, 0, 1) if not (dy == 0 and dx == 0)]
SS = 116  # psum region stride within a bank
PP = 64   # pool scratch block stride
N_FILLERS = int(os.environ.get("K_FILL", "3"))


def _window(t, Dr, Dc, seeds):
    rlo, rhi, clo, chi = seeds
    if t % 2 == 1:   # phase A: window over grid rows
        return max(0, rlo - t), min(Dr - 1, rhi + t)
    else:            # phase B: window over grid cols
        return max(0, clo - t), min(Dc - 1, chi + t)


def _K_of(t, Dr, Dc, seeds):
    # quantize K to a few long uniform phases: shape changes between
    # consecutive matmuls serialize the PE weight loads on HW, so uniformity
    # beats tight windows.
    lo, hi = _window(t, Dr, Dc, seeds)
    cap = (Dr if t % 2 == 1 else Dc) + 2
    if os.environ.get("K_WINDOWED", ""):
        need = hi + 3
        for step in (64, 96):
            if need <= step <= cap:
                return step
    return cap


def _cyc_Ks(Dr, Dc, seeds, num_sweeps):
    ks = {_K_of(t, Dr, Dc, seeds) for t in range(1, num_sweeps + 1)}
    ks.add(Dr + 2)
    ks.add(Dc + 2)
    return sorted(ks)


def build_program(Dr, Dc, seed_rlo, seed_rhi, seed_clo, seed_chi, r0, c0,
                  H, W, num_sweeps):
    """Domain = grid rows r0..r0+Dr-1, cols c0..c0+Dc-1; seed_* in domain coords."""
    Sr, Sc = Dr + 2, Dc + 2
    seeds = (seed_rlo, seed_rhi, seed_clo, seed_chi)
    assert Dr + 2 <= 128 and Dc + 2 <= 128 and 3 * SS <= 1536
    Ks = _cyc_Ks(Dr, Dc, seeds, num_sweeps)

    nc = bacc.Bacc("TRN2", target_bir_lowering=False, debug=False)

    # ---- DRAM I/O (inputs packed: single DMA) ----
    seg = [("obsT", Sr), ("obsTm", Sr), ("obsTp", Sr), ("xcT", Sr), ("xcTm", Sr),
           ("xcTp", Sr), ("ycT", Sr), ("startm", Dc), ("ident", max(Sc, Sr))]
    for K in Ks:
        seg.append((f"cycm1_{K}", K))
        seg.append((f"cycp1_{K}", K))
    offs, TOT = {}, 0
    for nm, wd in seg:
        offs[nm] = TOT
        TOT += wd
    NPACK = max(Sc, Sr, Dr + 2, Dc + 2)
    d_pack = nc.dram_tensor("packed", [NPACK, TOT], F32, kind="ExternalInput")
    d_out = nc.dram_tensor("out", [H, W], F32, kind="ExternalOutput")

    with tile.TileContext(nc) as tc:
        from contextlib import ExitStack
        with ExitStack() as ctx:
            sb = ctx.enter_context(tc.tile_pool(name="sb", bufs=1))
            ps = ctx.enter_context(tc.tile_pool(name="ps", bufs=1, space="PSUM"))

            # ---- SBUF tiles ----
            t_all = sb.tile([NPACK, TOT], F32)
            t_in = {nm: t_all[0:Sc, offs[nm]:offs[nm] + Sr] for nm in
                    ("obsT", "obsTm", "obsTp", "xcT", "xcTm", "xcTp", "ycT")}
            t_start = t_all[0:Dr, offs["startm"]:offs["startm"] + Dc]
            IDW = max(Sc, Sr)
            identC = sb.tile([IDW, IDW], F32)

            t_cyc = {}
            for Kv in Ks:
                t_cyc[(Kv, -1)] = sb.tile([Kv, Kv], F32, name=f"cycm1_{Kv}")
                t_cyc[(Kv, 1)] = sb.tile([Kv, Kv], F32, name=f"cycp1_{Kv}")

            def cycM(K, d):
                if d == 0:
                    return _rc(identC[0:K, 0:K])
                return _rc(t_cyc[(K, d)][:])

            # states, base-0 partitions with INF junk guards at the top two
            # partitions; free f = (col|row) f-1 with INF guard cells at both ends
            g_rm = sb.tile([Dr + 2, Dc + 2], F32)   # part p = row p
            s_T = sb.tile([Dc + 2, Dr + 2], F32)    # part p = col p
            bg = sb.tile([128, W], F32)
            bias_eps = sb.tile([Sc, 1], F32)
            sq = {nm: sb.tile([Sc, Dr], F32, name=f"sq_{nm}") for nm in ("L", "R", "U", "D")}
            t_tmp = sb.tile([Sc, Dr], F32)
            t_A = {ch: sb.tile([Sc, Dr], F32, name=f"A_{ch[0]+1}{ch[1]+1}") for ch in CHANNELS}
            t_mx = {ch: sb.tile([Sc, Dr], F32, name=f"mx_{ch[0]+1}{ch[1]+1}") for ch in CHANNELS}
            # cmap transposed: partition p = padded col p (grid col p-1); free = row
            t_cmapT = {ch: sb.tile([Sc, Dr], F32, name=f"cmapT_{ch[0]+1}{ch[1]+1}")
                       for ch in CHANNELS}
            # cmap row-major: partition p = row p (base 0); free = col
            t_cmapR = {ch: sb.tile([Dr + 2, Dc], F32, name=f"cmapR_{ch[0]+1}{ch[1]+1}")
                       for ch in CHANNELS}
            # pool-engine min-tree scratch + its output half
            t_pp = sb.tile([128, 3 * PP], F32)
            t_po = sb.tile([128, PP], F32)

            # ---- PSUM: two bank sets of 3 banks (3 regions each) ----
            psum_sets = [ps.tile([128, 1536], F32, name="psumA"),
                         ps.tile([128, 1536], F32, name="psumB")]
            psD = ps.tile([128, 512], F32, name="psD")
            t_warm = sb.tile([128, 512], mybir.dt.bfloat16)

            # ---- load inputs (single DMA) + const copies ----
            nc.sync.dma_start(t_all[:], d_pack.ap())
            v = nc.vector
            v.tensor_copy(identC[:], t_all[0:IDW, offs["ident"]:offs["ident"] + IDW])
            for Kv in Ks:
                o = offs[f"cycm1_{Kv}"]
                v.tensor_copy(t_cyc[(Kv, -1)][:], t_all[0:Kv, o:o + Kv])
                o = offs[f"cycp1_{Kv}"]
                v.tensor_copy(t_cyc[(Kv, 1)][:], t_all[0:Kv, o:o + Kv])

            # ---- init ----
            v.memset(t_warm[:], 1.0)
            v.memset(bg[:], INF)
            v.memset(g_rm[:], INF)
            v.memset(s_T[:], INF)
            v.memset(bias_eps[:], EPS_F)
            for ch in CHANNELS:
                v.memset(t_cmapR[ch][:], INF)

            # ---- background writes (1e7 outside the domain) ----
            out_ap = d_out.ap()
            bg_rows = []
            if r0 > 0:
                bg_rows.append((0, r0))
            if r0 + Dr < H:
                bg_rows.append((r0 + Dr, H))
            for lo_, hi_ in bg_rows:
                r = lo_
                while r < hi_:
                    n = min(128, hi_ - r)
                    nc.sync.dma_start(out_ap[r:r + n, :], bg[0:n, :])
                    r += n
            if c0 > 0:
                nc.sync.dma_start(out_ap[r0:r0 + Dr, 0:c0], bg[0:Dr, 0:c0])
            if c0 + Dc < W:
                nc.sync.dma_start(out_ap[r0:r0 + Dr, c0 + Dc:W],
                                  bg[0:Dr, 0:W - c0 - Dc])

            # ---- cmap channels, computed in transposed orientation ----
            # inputs have partition p = padded col p (grid col p-1); outputs are
            # written base-0 (partition p = grid col p) via a 1-partition-up slice.
            rows = slice(1, 1 + Dr)
            v.tensor_sub(t_tmp[:], t_in["xcT"][:, rows], t_in["xcTm"][:, rows])
            v.tensor_mul(sq["L"][:], t_tmp[:], t_tmp[:])
            v.tensor_sub(t_tmp[:], t_in["xcT"][:, rows], t_in["xcTp"][:, rows])
            v.tensor_mul(sq["R"][:], t_tmp[:], t_tmp[:])
            v.tensor_sub(t_tmp[:], t_in["ycT"][:, rows], t_in["ycT"][:, 2:2 + Dr])
            v.tensor_mul(sq["U"][:], t_tmp[:], t_tmp[:])
            v.tensor_sub(t_tmp[:], t_in["ycT"][:, rows], t_in["ycT"][:, 0:Dr])
            v.tensor_mul(sq["D"][:], t_tmp[:], t_tmp[:])

            geo = {(-1, -1): ("L", "U"), (0, -1): ("L",), (1, -1): ("L", "D"),
                   (-1, 0): ("U",), (1, 0): ("D",),
                   (-1, 1): ("R", "U"), (0, 1): ("R",), (1, 1): ("R", "D")}
            obsnb = {(-1, -1): (-1, -1), (0, -1): (-1, 0), (1, -1): (1, -1),
                     (-1, 0): (-1, 0), (1, 0): (1, 0),
                     (-1, 1): (-1, 1), (0, 1): (0, 1), (1, 1): (1, 1)}
            obs_by_dx = {-1: "obsTm", 0: "obsT", 1: "obsTp"}
            for ch in CHANNELS:
                terms = geo[ch]
                if len(terms) == 2:
                    v.tensor_add(t_A[ch][:], sq[terms[0]][:], sq[terms[1]][:])
                    nc.scalar.activation(t_A[ch][:], t_A[ch][:], ACTF.Sqrt,
                                         bias=bias_eps[:], scale=1.0)
                else:
                    nc.scalar.activation(t_A[ch][:], sq[terms[0]][:], ACTF.Sqrt,
                                         bias=bias_eps[:], scale=1.0)
                ody, odx = obsnb[ch]
                nbt = t_in[obs_by_dx[odx]]
                v.tensor_max(t_mx[ch][:], nbt[:, 1 + ody:1 + ody + Dr],
                             t_in["obsT"][:, rows])
                v.scalar_tensor_tensor(t_cmapT[ch][:, 0:Dr], t_mx[ch][:], OC,
                                       t_A[ch][:], op0=ALU.mult, op1=ALU.add)

            # ---- produce row-major cmap via setup transposes (identity rhs) ----
            for ch in CHANNELS:
                scratch = psum_sets[1][0:Dr, 0:Sc]
                nc.tensor.matmul(_rc(scratch), lhsT=_rc(t_cmapT[ch][0:Sc, 0:Dr]),
                                 rhs=cycM(Sc, 1),
                                 is_transpose=True, start=True, stop=True)
                v.tensor_copy(t_cmapR[ch][0:Dr, :], scratch[:, 0:Dc])

            # ---- g0 = clip(INF*(1-start), 0, INF) ----
            v.tensor_scalar(g_rm[0:Dr, 1:1 + Dc], t_start[:], -float(INF), float(INF),
                            op0=ALU.mult, op1=ALU.add)
            v.tensor_scalar_max(g_rm[0:Dr, 1:1 + Dc], g_rm[0:Dr, 1:1 + Dc], 0.0)

            # ---- helpers ----
            def ap3(tile_ap, col_off, dims):
                base = tile_ap
                pap = list(base.ap)
                return bass.AP(base.tensor, base.offset + col_off,
                               [list(pap[0])] + [list(d) for d in dims])

            def pool_split(lo, wlen):
                # DVE path: 9.4*nd + 1.04*np + overheads; Pool path must end
                # before the DVE reduce does: 11.1*np + sem <= 9.4*nd
                if wlen < 32 or not USE_POOL:
                    return wlen, 0
                nd = min(wlen, int((11.1 * wlen + 140.0) / 20.5) + 1)
                return nd, wlen - nd

            PRE_CHANS = [(-1, -1), (-1, 0), (-1, 1), (0, -1), (0, 1),
                         (1, -1), (1, 0), (1, 1)]
            BANK_FIRST = (0, 3, 5)   # idx that opens each psum bank's group

            def preload_A(set_idx, K, c0=0, c1=8):
                # cmap for an odd (g_rm -> s_T) sweep: column-major psum layout
                for idx in range(c0, c1):
                    dy, dx = PRE_CHANS[idx]
                    off = (dy + 1) * 512 + (dx + 1) * SS
                    nc.tensor.matmul(
                        _rc(psum_sets[set_idx][0:Dc, off:off + K]),
                        lhsT=_rc(t_cmapR[(dy, dx)][0:K, 0:Dc]),
                        rhs=_rc(identC[0:K, 0:K]),
                        is_transpose=True, start=(idx in BANK_FIRST), stop=False)

            def preload_B(set_idx, K, c0=0, c1=8):
                # cmap for an even (s_T -> g_rm) sweep: row-major psum layout
                for idx in range(c0, c1):
                    dy, dx = PRE_CHANS[idx]
                    off = (dy + 1) * 512 + (dx + 1) * SS
                    nc.tensor.matmul(
                        _rc(psum_sets[set_idx][0:Dr, off:off + K]),
                        lhsT=_rc(t_cmapT[(dy, dx)][0:K, 0:Dr]),
                        rhs=cycM(K, 1),
                        is_transpose=True, start=(idx in BANK_FIRST), stop=False)

            def emit_fillers(n, K):
                for _ in range(n):
                    nc.tensor.matmul(psD[0:K, 0:K], lhsT=identC[0:K, 0:K],
                                     rhs=identC[0:K, 0:K], is_transpose=True,
                                     start=True, stop=True,
                                     skip_group_check=True)

            preload_A(0, _K_of(1, Dr, Dc, seeds))

            # ---- sweeps ----
            for t in range(1, num_sweeps + 1):
                cur = psum_sets[(t - 1) % 2]
                lo, hi = _window(t, Dr, Dc, seeds)
                K = _K_of(t, Dr, Dc, seeds)
                wlen = hi - lo + 1
                nd, np_ = pool_split(lo, wlen)
                m = lo + nd
                if t % 2 == 1:
                    # phase A: g_rm -> s_T; windowed over rows
                    NPART = Dc
                    dst = s_T
                    for dy in (-1, 0, 1):
                        for dx in (-1, 0, 1):
                            off = (dy + 1) * 512 + (dx + 1) * SS
                            nc.tensor.matmul(
                                _rc(cur[0:Dc, off:off + K]),
                                lhsT=_rc(g_rm[0:K, dx + 1:dx + 1 + Dc]),
                                rhs=cycM(K, dy),
                                is_transpose=True, start=False, stop=(dx == 1))
                else:
                    # phase B: s_T -> g_rm; windowed over cols
                    NPART = Dr
                    dst = g_rm
                    for dy in (-1, 0, 1):
                        for dx in (-1, 0, 1):
                            off = (dy + 1) * 512 + (dx + 1) * SS
                            nc.tensor.matmul(
                                _rc(cur[0:Dr, off:off + K]),
                                lhsT=_rc(s_T[0:K, dy + 1:dy + 1 + Dr]),
                                rhs=cycM(K, dx),
                                is_transpose=True, start=False, stop=(dx == 1))

                if t < num_sweeps:
                    K2 = _K_of(t + 1, Dr, Dc, seeds)
                    # fillers only where the sweep is chain-bound (big
                    # reduce): early small-window sweeps are stream-bound and
                    # the PE never idles there.
                    nfill = N_FILLERS if wlen > 90 else (2 if wlen > 64 else 0)
                    if t % 2 == 1:
                        preload_B(t % 2, K2, 0, 4)
                        emit_fillers(nfill, K2)
                        preload_B(t % 2, K2, 4, 8)
                    else:
                        preload_A(t % 2, K2, 0, 4)
                        emit_fillers(nfill, K2)
                        preload_A(t % 2, K2, 4, 8)

                base = cur[0:NPART, 0:1536]
                if np_ > 0:
                    # pool computes its half into its own tile (no state-tile
                    # write ordering vs the DVE reduce); DVE merges after.
                    gp = nc.gpsimd
                    scr = t_pp[0:NPART, :]
                    i0 = ap3(base, 0 * SS + m, [[512, 3], [1, np_]])
                    i2 = ap3(base, 2 * SS + m, [[512, 3], [1, np_]])
                    o1 = ap3(scr, 0, [[PP, 3], [1, np_]])
                    gp.tensor_tensor(o1, i0, i2, op=ALU.min)
                    i1 = ap3(base, 1 * SS + m, [[512, 3], [1, np_]])
                    gp.tensor_tensor(o1, o1, i1, op=ALU.min)
                    a = ap3(scr, 0 * PP, [[1, np_]])
                    b = ap3(scr, 2 * PP, [[1, np_]])
                    c = ap3(scr, 1 * PP, [[1, np_]])
                    gp.tensor_tensor(b, a, b, op=ALU.min)
                    gp.tensor_tensor(t_po[0:NPART, 0:np_], c, b, op=ALU.min)
                if nd > 0:
                    in_ap = ap3(base, lo, [[1, nd], [512, 3], [SS, 3]])
                    v.tensor_reduce(dst[0:NPART, 1 + lo:1 + lo + nd], in_ap,
                                    axis=AXL.XY, op=ALU.min)
                if np_ > 0:
                    v.tensor_copy(dst[0:NPART, 1 + m:1 + m + np_],
                                  t_po[0:NPART, 0:np_])

            # ---- final state to row-major if needed, then write out ----
            if num_sweeps % 2 == 1:
                fin = psum_sets[num_sweeps % 2][0:Dr, 0:Dc]
                nc.tensor.matmul(fin, lhsT=s_T[0:Dc, 1:1 + Dr],
                                 rhs=identC[0:Dc, 0:Dc],
                                 is_transpose=True, start=True, stop=True)
                v.tensor_copy(g_rm[0:Dr, 1:1 + Dc], fin)
            nc.sync.dma_start(out_ap[r0:r0 + Dr, c0:c0 + Dc],
                              g_rm[0:Dr, 1:1 + Dc])

    nc.compile()
    return nc, ["packed"]


def prep_inputs(obstacles, coords, start_map, num_sweeps=NUM_SWEEPS):
    """Host-side slicing/layout prep. Returns (in_map, geometry)."""
    obs = np.asarray(obstacles, np.float32)[0, 0]
    yc = np.asarray(coords, np.float32)[0, 0]
    xc = np.asarray(coords, np.float32)[0, 1]
    s = np.asarray(start_map, np.float32)[0, 0]
    H, W = obs.shape

    ys, xs = np.nonzero(s > 0)
    assert len(ys) >= 1, "empty start_map"
    r0 = max(0, int(ys.min()) - num_sweeps)
    r1 = min(H - 1, int(ys.max()) + num_sweeps)
    c0 = max(0, int(xs.min()) - num_sweeps)
    c1 = min(W - 1, int(xs.max()) + num_sweeps)
    Dr, Dc = r1 - r0 + 1, c1 - c0 + 1
    Sr, Sc = Dr + 2, Dc + 2
    seeds = (int(ys.min()) - r0, int(ys.max()) - r0,
             int(xs.min()) - c0, int(xs.max()) - c0)
    Ks = _cyc_Ks(Dr, Dc, seeds, num_sweeps)
    NPACK = max(Sc, Sr, Dr + 2, Dc + 2)

    def pad_slice(a):
        ap = np.pad(a, 1, mode='edge')
        return np.ascontiguousarray(ap[r0:r0 + Sr, c0:c0 + Sc], dtype=np.float32)

    obs_p, yc_p, xc_p = pad_slice(obs), pad_slice(yc), pad_slice(xc)

    def tsh(a, dx):
        at = np.ascontiguousarray(a.T)
        if dx == 0:
            return at
        out = np.empty_like(at)
        if dx == -1:
            out[1:] = at[:-1]
            out[0] = at[0]
        else:
            out[:-1] = at[1:]
            out[-1] = at[-1]
        return out

    def cyc(n, d):
        # P[k, j] = 1 iff k == (j + d) mod n
        P = np.zeros((n, n), np.float32)
        P[(np.arange(n) + d) % n, np.arange(n)] = 1.0
        return P

    def frame(a, pw):
        out = np.zeros((NPACK, pw), np.float32)
        out[0:a.shape[0], 0:a.shape[1]] = a
        return out

    IDW = max(Sc, Sr)
    startm = np.zeros((Sc, Dc), np.float32)
    startm[0:Dr, :] = s[r0:r1 + 1, c0:c1 + 1]
    parts = [
        tsh(obs_p, 0), tsh(obs_p, -1), tsh(obs_p, 1),
        tsh(xc_p, 0), tsh(xc_p, -1), tsh(xc_p, 1), tsh(yc_p, 0),
        startm, np.eye(IDW, dtype=np.float32),
    ]
    parts = [frame(a, a.shape[1]) for a in parts]
    for K in Ks:
        parts.append(frame(cyc(K, -1), K))
        parts.append(frame(cyc(K, 1), K))
    packed = np.concatenate(parts, axis=1)
    in_map = {"packed": np.ascontiguousarray(packed, dtype=np.float32)}

    geom = dict(Dr=Dr, Dc=Dc, r0=r0, c0=c0, H=H, W=W,
                seed_rlo=seeds[0], seed_rhi=seeds[1],
                seed_clo=seeds[2], seed_chi=seeds[3])
    return in_map, geom


def kernel(obstacles, coords, start_map, goal_map):
    in_map, gm = prep_inputs(obstacles, coords, start_map)
    nc, _ = build_program(gm["Dr"], gm["Dc"], gm["seed_rlo"], gm["seed_rhi"],
                          gm["seed_clo"], gm["seed_chi"], gm["r0"], gm["c0"],
                          gm["H"], gm["W"], NUM_SWEEPS)
    in_maps = [in_map for _ in range(N_CORES)]
    res = run_bass_kernel_spmd(nc, in_maps, core_ids=list(range(N_CORES)))
    out = res.results[0]["out"]
    return np.ascontiguousarray(out[None, None]).astype(np.float32)



# revision 13
# speedup vs baseline: 1.2045x; 1.2045x over previous
"""Trainium2 Bass kernel for nn_DifferentiableStarPlanner.

Algorithm notes (validated bitwise vs the reference in numpy):

  * The reference's open/close/pool computations never feed the returned
    tensor: the output is exactly NUM_SWEEPS Jacobi sweeps of a 9-channel
    min-plus stencil  g <- min(g, min_c(shift_c(g) + cmap_c))  with
    g0 = 1e7 everywhere except the start cell.
  * Only the start bounding box inflated by NUM_SWEEPS (clipped) can change
    from 1e7: a 113x113 corner here.  Edge-replicate padding is replaced by
    1e7 guard cells (provably never the argmin), the center channel by a
    pure-copy identity channel.
  * Per sweep only cells within t steps of the start can change, so all
    per-sweep work is windowed to the active wavefront.

Device mapping (one NeuronCore; all 8 cores run identical replicas).
The state alternates orientation every sweep; every neighbor shift is ONE
TensorEngine transpose-mode matmul (pure routing, bit-exact):

  * transpose-mode semantics: out[m, j] = lhsT[perm(j), m].  The stationary
    lhsT is the state sliced [0:K] (PE requires base partition 0); perm is a
    K x K cyclic permutation encoding the +-1 row shift (identity for the
    -1 shift); the col shift is the free-dim base of the lhsT slice.  INF
    guard partitions/columns at both ends of the state supply the domain
    boundary; cyclic wraps land in junk output columns beyond the window.
  * K = (window_hi + 3) rounded up to a multiple of 8 (a handful of distinct
    cyclic matrices, shipped in the packed input).  Matmul cost scales with
    output free size = K, so shift AND cmap-preload matmuls track the
    wavefront instead of paying full width every sweep.
  * Accumulation onto the cmap preload uses PSUM first-touch-overwrite
    semantics; the center region is never preloaded so the identity channel
    injects g itself.
  * The 9-way min reduce is split by columns between the DVE (one
    tensor_reduce) and the Pool/gpsimd engine (a 4-instruction min tree),
    halving the reduce latency on the critical path.
  * cmap for sweep t+1 is preloaded into the other PSUM bank set during
    sweep t, overlapped with the reduce.
"""
import sys
import os
import numpy as np

for _p in ("/opt/trn_rl_repo", "/root/.axon_site/_ro/trn_rl_repo"):
    if os.path.isdir(_p) and _p not in sys.path:
        sys.path.insert(0, _p)

import concourse.bass as bass
import concourse.bacc as bacc
import concourse.mybir as mybir
from concourse import tile
from concourse.bass_utils import run_bass_kernel_spmd

F32 = mybir.dt.float32
F32R = mybir.dt.float32r
F16 = mybir.dt.float16

USE_F32 = os.environ.get("K_F32", "") != ""
USE_F32R = os.environ.get("K_F32R", "") != ""
USE_POOL = os.environ.get("K_POOL", "") != ""

# fp16 mode: state/cmap/perm matrices in fp16 at scale 2^-10 (transpose
# matmuls run 1 cyc/row vs 2 for fp32); PSUM accumulate of fp16 payloads.
DT = F32 if (USE_F32 or USE_F32R) else F16
SCALE = 1.0 if DT is F32 else float(np.float32(2.0 ** -10))
PS_BANK = 512 if DT is F32 else 1024  # psum bank stride, elements


def _rc(ap):
    return ap.bitcast(F32R) if USE_F32R else ap
ALU = mybir.AluOpType
AXL = mybir.AxisListType
ACTF = mybir.ActivationFunctionType

INF = np.float32(1.0e7)
OC = float(np.float32(10000.0))
EPS_F = np.float32(1e-12)
NUM_SWEEPS = 80
N_CORES = 8

# channels: (dy, dx), center excluded
CHANNELS = [(dy, dx) for dy in (-1, 0, 1) for dx in (-1, 0, 1) if not (dy == 0 and dx == 0)]
SS = 116 if DT is F32 else 232  # psum region stride within a bank, elements
PP = 64   # pool scratch block stride
N_FILLERS = int(os.environ.get("K_FILL", "3"))


def _window(t, Dr, Dc, seeds):
    rlo, rhi, clo, chi = seeds
    if t % 2 == 1:   # phase A: window over grid rows
        return max(0, rlo - t), min(Dr - 1, rhi + t)
    else:            # phase B: window over grid cols
        return max(0, clo - t), min(Dc - 1, chi + t)


def _K_of(t, Dr, Dc, seeds):
    # quantize K to a few long uniform phases: shape changes between
    # consecutive matmuls serialize the PE weight loads on HW, so uniformity
    # beats tight windows.
    lo, hi = _window(t, Dr, Dc, seeds)
    cap = (Dr if t % 2 == 1 else Dc) + 2
    if os.environ.get("K_WINDOWED", ""):
        need = hi + 3
        for step in (64, 96):
            if need <= step <= cap:
                return step
    return cap


def _cyc_Ks(Dr, Dc, seeds, num_sweeps):
    ks = {_K_of(t, Dr, Dc, seeds) for t in range(1, num_sweeps + 1)}
    ks.add(Dr + 2)
    ks.add(Dc + 2)
    return sorted(ks)


def build_program(Dr, Dc, seed_rlo, seed_rhi, seed_clo, seed_chi, r0, c0,
                  H, W, num_sweeps):
    """Domain = grid rows r0..r0+Dr-1, cols c0..c0+Dc-1; seed_* in domain coords."""
    Sr, Sc = Dr + 2, Dc + 2
    seeds = (seed_rlo, seed_rhi, seed_clo, seed_chi)
    assert Dr + 2 <= 128 and Dc + 2 <= 128 and 3 * SS <= 3 * PS_BANK
    Ks = _cyc_Ks(Dr, Dc, seeds, num_sweeps)

    nc = bacc.Bacc("TRN2", target_bir_lowering=False, debug=False)

    # ---- DRAM I/O (inputs packed: single DMA) ----
    seg = [("obsT", Sr), ("obsTm", Sr), ("obsTp", Sr), ("xcT", Sr), ("xcTm", Sr),
           ("xcTp", Sr), ("ycT", Sr), ("startm", Dc), ("ident", max(Sc, Sr))]
    for K in Ks:
        seg.append((f"cycm1_{K}", K))
        seg.append((f"cycp1_{K}", K))
    offs, TOT = {}, 0
    for nm, wd in seg:
        offs[nm] = TOT
        TOT += wd
    NPACK = max(Sc, Sr, Dr + 2, Dc + 2)
    d_pack = nc.dram_tensor("packed", [NPACK, TOT], F32, kind="ExternalInput")
    d_out = nc.dram_tensor("out", [H, W], F32, kind="ExternalOutput")

    with tile.TileContext(nc) as tc:
        from contextlib import ExitStack
        with ExitStack() as ctx:
            sb = ctx.enter_context(tc.tile_pool(name="sb", bufs=1))
            ps = ctx.enter_context(tc.tile_pool(name="ps", bufs=1, space="PSUM"))

            # ---- SBUF tiles ----
            t_all = sb.tile([NPACK, TOT], F32)
            t_in = {nm: t_all[0:Sc, offs[nm]:offs[nm] + Sr] for nm in
                    ("obsT", "obsTm", "obsTp", "xcT", "xcTm", "xcTp", "ycT")}
            t_start = t_all[0:Dr, offs["startm"]:offs["startm"] + Dc]
            IDW = max(Sc, Sr)
            identC = sb.tile([IDW, IDW], DT)

            t_cyc = {}
            for Kv in Ks:
                t_cyc[(Kv, -1)] = sb.tile([Kv, Kv], DT, name=f"cycm1_{Kv}")
                t_cyc[(Kv, 1)] = sb.tile([Kv, Kv], DT, name=f"cycp1_{Kv}")

            def cycM(K, d):
                if d == 0:
                    return _rc(identC[0:K, 0:K])
                return _rc(t_cyc[(K, d)][:])

            # states, base-0 partitions with INF junk guards at the top two
            # partitions; free f = (col|row) f-1 with INF guard cells at both ends
            g_rm = sb.tile([Dr + 2, Dc + 2], DT)   # part p = row p
            s_T = sb.tile([Dc + 2, Dr + 2], DT)    # part p = col p
            bg = sb.tile([128, W], F32)
            bias_eps = sb.tile([Sc, 1], F32)
            sq = {nm: sb.tile([Sc, Dr], F32, name=f"sq_{nm}") for nm in ("L", "R", "U", "D")}
            t_tmp = sb.tile([Sc, Dr], F32)
            t_A = {ch: sb.tile([Sc, Dr], F32, name=f"A_{ch[0]+1}{ch[1]+1}") for ch in CHANNELS}
            t_mx = {ch: sb.tile([Sc, Dr], F32, name=f"mx_{ch[0]+1}{ch[1]+1}") for ch in CHANNELS}
            # cmap transposed: partition p = padded col p (grid col p-1); free = row
            t_cmapT = {ch: sb.tile([Sc, Dr], DT, name=f"cmapT_{ch[0]+1}{ch[1]+1}")
                       for ch in CHANNELS}
            # cmap row-major: partition p = row p (base 0); free = col
            t_cmapR = {ch: sb.tile([Dr + 2, Dc], DT, name=f"cmapR_{ch[0]+1}{ch[1]+1}")
                       for ch in CHANNELS}
            # pool-engine min-tree scratch + its output half
            t_pp = sb.tile([128, 3 * PP], DT)
            t_po = sb.tile([128, PP], DT)
            # full-precision output staging (state is scaled DT)
            t_fin = sb.tile([Dr, Dc], F32)

            # ---- PSUM: two bank sets of 3 banks (3 regions each) ----
            psum_sets = [ps.tile([128, 3 * PS_BANK], DT, name="psumA"),
                         ps.tile([128, 3 * PS_BANK], DT, name="psumB")]
            psD = ps.tile([128, PS_BANK], DT, name="psD")

            # ---- load inputs (single DMA) + const copies ----
            nc.sync.dma_start(t_all[:], d_pack.ap())
            v = nc.vector
            v.tensor_copy(identC[:], t_all[0:IDW, offs["ident"]:offs["ident"] + IDW])
            for Kv in Ks:
                o = offs[f"cycm1_{Kv}"]
                v.tensor_copy(t_cyc[(Kv, -1)][:], t_all[0:Kv, o:o + Kv])
                o = offs[f"cycp1_{Kv}"]
                v.tensor_copy(t_cyc[(Kv, 1)][:], t_all[0:Kv, o:o + Kv])

            # ---- init ----
            v.memset(bg[:], INF)
            v.memset(g_rm[:], INF * SCALE)
            v.memset(s_T[:], INF * SCALE)
            v.memset(bias_eps[:], EPS_F * SCALE * SCALE)
            for ch in CHANNELS:
                v.memset(t_cmapR[ch][:], INF * SCALE)

            # ---- background writes (1e7 outside the domain) ----
            out_ap = d_out.ap()
            bg_rows = []
            if r0 > 0:
                bg_rows.append((0, r0))
            if r0 + Dr < H:
                bg_rows.append((r0 + Dr, H))
            for lo_, hi_ in bg_rows:
                r = lo_
                while r < hi_:
                    n = min(128, hi_ - r)
                    nc.sync.dma_start(out_ap[r:r + n, :], bg[0:n, :])
                    r += n
            if c0 > 0:
                nc.sync.dma_start(out_ap[r0:r0 + Dr, 0:c0], bg[0:Dr, 0:c0])
            if c0 + Dc < W:
                nc.sync.dma_start(out_ap[r0:r0 + Dr, c0 + Dc:W],
                                  bg[0:Dr, 0:W - c0 - Dc])

            # ---- cmap channels, computed in transposed orientation ----
            # inputs have partition p = padded col p (grid col p-1); outputs are
            # written base-0 (partition p = grid col p) via a 1-partition-up slice.
            rows = slice(1, 1 + Dr)
            v.tensor_sub(t_tmp[:], t_in["xcT"][:, rows], t_in["xcTm"][:, rows])
            v.tensor_mul(sq["L"][:], t_tmp[:], t_tmp[:])
            v.tensor_sub(t_tmp[:], t_in["xcT"][:, rows], t_in["xcTp"][:, rows])
            v.tensor_mul(sq["R"][:], t_tmp[:], t_tmp[:])
            v.tensor_sub(t_tmp[:], t_in["ycT"][:, rows], t_in["ycT"][:, 2:2 + Dr])
            v.tensor_mul(sq["U"][:], t_tmp[:], t_tmp[:])
            v.tensor_sub(t_tmp[:], t_in["ycT"][:, rows], t_in["ycT"][:, 0:Dr])
            v.tensor_mul(sq["D"][:], t_tmp[:], t_tmp[:])

            geo = {(-1, -1): ("L", "U"), (0, -1): ("L",), (1, -1): ("L", "D"),
                   (-1, 0): ("U",), (1, 0): ("D",),
                   (-1, 1): ("R", "U"), (0, 1): ("R",), (1, 1): ("R", "D")}
            obsnb = {(-1, -1): (-1, -1), (0, -1): (-1, 0), (1, -1): (1, -1),
                     (-1, 0): (-1, 0), (1, 0): (1, 0),
                     (-1, 1): (-1, 1), (0, 1): (0, 1), (1, 1): (1, 1)}
            obs_by_dx = {-1: "obsTm", 0: "obsT", 1: "obsTp"}
            for ch in CHANNELS:
                terms = geo[ch]
                if len(terms) == 2:
                    v.tensor_add(t_A[ch][:], sq[terms[0]][:], sq[terms[1]][:])
                    nc.scalar.activation(t_A[ch][:], t_A[ch][:], ACTF.Sqrt,
                                         bias=bias_eps[:], scale=SCALE * SCALE)
                else:
                    nc.scalar.activation(t_A[ch][:], sq[terms[0]][:], ACTF.Sqrt,
                                         bias=bias_eps[:], scale=SCALE * SCALE)
                ody, odx = obsnb[ch]
                nbt = t_in[obs_by_dx[odx]]
                v.tensor_max(t_mx[ch][:], nbt[:, 1 + ody:1 + ody + Dr],
                             t_in["obsT"][:, rows])
                v.scalar_tensor_tensor(t_cmapT[ch][:, 0:Dr], t_mx[ch][:], OC * SCALE,
                                       t_A[ch][:], op0=ALU.mult, op1=ALU.add)

            # ---- produce row-major cmap via setup transposes (identity rhs) ----
            for ch in CHANNELS:
                scratch = psum_sets[1][0:Dr, 0:Sc]
                nc.tensor.matmul(_rc(scratch), lhsT=_rc(t_cmapT[ch][0:Sc, 0:Dr]),
                                 rhs=cycM(Sc, 1),
                                 is_transpose=True, start=True, stop=True)
                v.tensor_copy(t_cmapR[ch][0:Dr, :], scratch[:, 0:Dc])

            # ---- g0 = clip(INF*(1-start), 0, INF), scaled ----
            v.tensor_scalar(g_rm[0:Dr, 1:1 + Dc], t_start[:], -float(INF) * SCALE,
                            float(INF) * SCALE, op0=ALU.mult, op1=ALU.add)
            v.tensor_scalar_max(g_rm[0:Dr, 1:1 + Dc], g_rm[0:Dr, 1:1 + Dc], 0.0)

            # ---- helpers ----
            def ap3(tile_ap, col_off, dims):
                base = tile_ap
                pap = list(base.ap)
                return bass.AP(base.tensor, base.offset + col_off,
                               [list(pap[0])] + [list(d) for d in dims])

            def pool_split(lo, wlen):
                # DVE path: 9.4*nd + 1.04*np + overheads; Pool path must end
                # before the DVE reduce does: 11.1*np + sem <= 9.4*nd
                if wlen < 32 or not USE_POOL:
                    return wlen, 0
                nd = min(wlen, int((11.1 * wlen + 140.0) / 20.5) + 1)
                return nd, wlen - nd

            PRE_CHANS = [(-1, -1), (-1, 0), (-1, 1), (0, -1), (0, 1),
                         (1, -1), (1, 0), (1, 1)]
            BANK_FIRST = (0, 3, 5)   # idx that opens each psum bank's group

            def preload_A(set_idx, K, c0=0, c1=8):
                # cmap for an odd (g_rm -> s_T) sweep: column-major psum layout
                for idx in range(c0, c1):
                    dy, dx = PRE_CHANS[idx]
                    off = (dy + 1) * PS_BANK + (dx + 1) * SS
                    nc.tensor.matmul(
                        _rc(psum_sets[set_idx][0:Dc, off:off + K]),
                        lhsT=_rc(t_cmapR[(dy, dx)][0:K, 0:Dc]),
                        rhs=_rc(identC[0:K, 0:K]),
                        is_transpose=True, start=(idx in BANK_FIRST), stop=False)

            def preload_B(set_idx, K, c0=0, c1=8):
                # cmap for an even (s_T -> g_rm) sweep: row-major psum layout
                for idx in range(c0, c1):
                    dy, dx = PRE_CHANS[idx]
                    off = (dy + 1) * PS_BANK + (dx + 1) * SS
                    nc.tensor.matmul(
                        _rc(psum_sets[set_idx][0:Dr, off:off + K]),
                        lhsT=_rc(t_cmapT[(dy, dx)][0:K, 0:Dr]),
                        rhs=cycM(K, 1),
                        is_transpose=True, start=(idx in BANK_FIRST), stop=False)

            def emit_fillers(n, K):
                for _ in range(n):
                    nc.tensor.matmul(psD[0:K, 0:K], lhsT=identC[0:K, 0:K],
                                     rhs=identC[0:K, 0:K], is_transpose=True,
                                     start=True, stop=True,
                                     skip_group_check=True)

            preload_A(0, _K_of(1, Dr, Dc, seeds))

            # ---- sweeps ----
            for t in range(1, num_sweeps + 1):
                cur = psum_sets[(t - 1) % 2]
                lo, hi = _window(t, Dr, Dc, seeds)
                K = _K_of(t, Dr, Dc, seeds)
                wlen = hi - lo + 1
                nd, np_ = pool_split(lo, wlen)
                m = lo + nd
                if t % 2 == 1:
                    # phase A: g_rm -> s_T; windowed over rows
                    NPART = Dc
                    dst = s_T
                    for dy in (-1, 0, 1):
                        for dx in (-1, 0, 1):
                            off = (dy + 1) * PS_BANK + (dx + 1) * SS
                            nc.tensor.matmul(
                                _rc(cur[0:Dc, off:off + K]),
                                lhsT=_rc(g_rm[0:K, dx + 1:dx + 1 + Dc]),
                                rhs=cycM(K, dy),
                                is_transpose=True, start=False, stop=(dx == 1))
                else:
                    # phase B: s_T -> g_rm; windowed over cols
                    NPART = Dr
                    dst = g_rm
                    for dy in (-1, 0, 1):
                        for dx in (-1, 0, 1):
                            off = (dy + 1) * PS_BANK + (dx + 1) * SS
                            nc.tensor.matmul(
                                _rc(cur[0:Dr, off:off + K]),
                                lhsT=_rc(s_T[0:K, dy + 1:dy + 1 + Dr]),
                                rhs=cycM(K, dx),
                                is_transpose=True, start=False, stop=(dx == 1))

                if t < num_sweeps:
                    K2 = _K_of(t + 1, Dr, Dc, seeds)
                    # fillers only where the sweep is chain-bound (big
                    # reduce): early small-window sweeps are stream-bound and
                    # the PE never idles there.
                    nfill = N_FILLERS if wlen > 90 else (2 if wlen > 64 else 0)
                    if t % 2 == 1:
                        preload_B(t % 2, K2, 0, 4)
                        emit_fillers(nfill, K2)
                        preload_B(t % 2, K2, 4, 8)
                    else:
                        preload_A(t % 2, K2, 0, 4)
                        emit_fillers(nfill, K2)
                        preload_A(t % 2, K2, 4, 8)

                base = cur[0:NPART, 0:3 * PS_BANK]
                if np_ > 0:
                    # pool computes its half into its own tile (no state-tile
                    # write ordering vs the DVE reduce); DVE merges after.
                    gp = nc.gpsimd
                    scr = t_pp[0:NPART, :]
                    i0 = ap3(base, 0 * SS + m, [[PS_BANK, 3], [1, np_]])
                    i2 = ap3(base, 2 * SS + m, [[PS_BANK, 3], [1, np_]])
                    o1 = ap3(scr, 0, [[PP, 3], [1, np_]])
                    gp.tensor_tensor(o1, i0, i2, op=ALU.min)
                    i1 = ap3(base, 1 * SS + m, [[PS_BANK, 3], [1, np_]])
                    gp.tensor_tensor(o1, o1, i1, op=ALU.min)
                    a = ap3(scr, 0 * PP, [[1, np_]])
                    b = ap3(scr, 2 * PP, [[1, np_]])
                    c = ap3(scr, 1 * PP, [[1, np_]])
                    gp.tensor_tensor(b, a, b, op=ALU.min)
                    gp.tensor_tensor(t_po[0:NPART, 0:np_], c, b, op=ALU.min)
                if nd > 0:
                    in_ap = ap3(base, lo, [[1, nd], [PS_BANK, 3], [SS, 3]])
                    v.tensor_reduce(dst[0:NPART, 1 + lo:1 + lo + nd], in_ap,
                                    axis=AXL.XY, op=ALU.min)
                if np_ > 0:
                    v.tensor_copy(dst[0:NPART, 1 + m:1 + m + np_],
                                  t_po[0:NPART, 0:np_])

            # ---- final state to row-major if needed, then write out ----
            if num_sweeps % 2 == 1:
                fin = psum_sets[num_sweeps % 2][0:Dr, 0:Dc]
                nc.tensor.matmul(fin, lhsT=s_T[0:Dc, 1:1 + Dr],
                                 rhs=identC[0:Dc, 0:Dc],
                                 is_transpose=True, start=True, stop=True)
                v.tensor_copy(g_rm[0:Dr, 1:1 + Dc], fin)
            v.tensor_scalar_mul(t_fin[:], g_rm[0:Dr, 1:1 + Dc], 1.0 / SCALE)
            nc.sync.dma_start(out_ap[r0:r0 + Dr, c0:c0 + Dc], t_fin[:])

    nc.compile()
    return nc, ["packed"]


def prep_inputs(obstacles, coords, start_map, num_sweeps=NUM_SWEEPS):
    """Host-side slicing/layout prep. Returns (in_map, geometry)."""
    obs = np.asarray(obstacles, np.float32)[0, 0]
    yc = np.asarray(coords, np.float32)[0, 0]
    xc = np.asarray(coords, np.float32)[0, 1]
    s = np.asarray(start_map, np.float32)[0, 0]
    H, W = obs.shape

    ys, xs = np.nonzero(s > 0)
    assert len(ys) >= 1, "empty start_map"
    r0 = max(0, int(ys.min()) - num_sweeps)
    r1 = min(H - 1, int(ys.max()) + num_sweeps)
    c0 = max(0, int(xs.min()) - num_sweeps)
    c1 = min(W - 1, int(xs.max()) + num_sweeps)
    Dr, Dc = r1 - r0 + 1, c1 - c0 + 1
    Sr, Sc = Dr + 2, Dc + 2
    seeds = (int(ys.min()) - r0, int(ys.max()) - r0,
             int(xs.min()) - c0, int(xs.max()) - c0)
    Ks = _cyc_Ks(Dr, Dc, seeds, num_sweeps)
    NPACK = max(Sc, Sr, Dr + 2, Dc + 2)

    def pad_slice(a):
        ap = np.pad(a, 1, mode='edge')
        return np.ascontiguousarray(ap[r0:r0 + Sr, c0:c0 + Sc], dtype=np.float32)

    obs_p, yc_p, xc_p = pad_slice(obs), pad_slice(yc), pad_slice(xc)

    def tsh(a, dx):
        at = np.ascontiguousarray(a.T)
        if dx == 0:
            return at
        out = np.empty_like(at)
        if dx == -1:
            out[1:] = at[:-1]
            out[0] = at[0]
        else:
            out[:-1] = at[1:]
            out[-1] = at[-1]
        return out

    def cyc(n, d):
        # P[k, j] = 1 iff k == (j + d) mod n
        P = np.zeros((n, n), np.float32)
        P[(np.arange(n) + d) % n, np.arange(n)] = 1.0
        return P

    def frame(a, pw):
        out = np.zeros((NPACK, pw), np.float32)
        out[0:a.shape[0], 0:a.shape[1]] = a
        return out

    IDW = max(Sc, Sr)
    startm = np.zeros((Sc, Dc), np.float32)
    startm[0:Dr, :] = s[r0:r1 + 1, c0:c1 + 1]
    parts = [
        tsh(obs_p, 0), tsh(obs_p, -1), tsh(obs_p, 1),
        tsh(xc_p, 0), tsh(xc_p, -1), tsh(xc_p, 1), tsh(yc_p, 0),
        startm, np.eye(IDW, dtype=np.float32),
    ]
    parts = [frame(a, a.shape[1]) for a in parts]
    for K in Ks:
        parts.append(frame(cyc(K, -1), K))
        parts.append(frame(cyc(K, 1), K))
    packed = np.concatenate(parts, axis=1)
    in_map = {"packed": np.ascontiguousarray(packed, dtype=np.float32)}

    geom = dict(Dr=Dr, Dc=Dc, r0=r0, c0=c0, H=H, W=W,
                seed_rlo=seeds[0], seed_rhi=seeds[1],
                seed_clo=seeds[2], seed_chi=seeds[3])
    return in_map, geom


def kernel(obstacles, coords, start_map, goal_map):
    in_map, gm = prep_inputs(obstacles, coords, start_map)
    nc, _ = build_program(gm["Dr"], gm["Dc"], gm["seed_rlo"], gm["seed_rhi"],
                          gm["seed_clo"], gm["seed_chi"], gm["r0"], gm["c0"],
                          gm["H"], gm["W"], NUM_SWEEPS)
    in_maps = [in_map for _ in range(N_CORES)]
    res = run_bass_kernel_spmd(nc, in_maps, core_ids=list(range(N_CORES)))
    out = res.results[0]["out"]
    return np.ascontiguousarray(out[None, None]).astype(np.float32)

